# revision 24
# baseline (speedup 1.0000x reference)
"""Trainium2 Bass kernel for nn_PhysicsResidual (WavePINN wave-equation residual).

Per collocation point p = (t,x,y,z):
    u = MLP_128x6_tanh(p)   (4 -> 128 -> 128 x5 -> 1, tanh, linear head)
    psi = MLP_32x2_tanh(p)  (4 -> 32 -> 32 -> 1)
    d_i = diag(Hessian u)[i],  lap = d1+d2+d3
    resid = d0 - (1+psi)^2 * lap

Algorithm (per point, exact AD):
  forward:  h_k = tanh(a_k), a_k = W_k h_{k-1} + b_k, sq_k = h_k^2
  backward (sign-flipped adjoints skip materializing D=1-h^2):
            vtn_6 = (sq_6-1)*wout, vtn_{k-1} = (sq_{k-1}-1)*(W_k^T vtn_k)
            so vtn_k = +-vt_k with per-layer parity; rn_k = h_k*vtn_k
  jets:     hj_{1,i} = (sq_1-1)*W1[:,i]; aj_{k,i} = W_k hj_{k-1,i};
            hj_{k,i} = (sq_k-1)*aj_{k,i}   (signs cancel in squares)
  d rows:   acc += w_k * ones^T (sqj_k * rn_k), w_k = +-2 by layer parity
            (PSUM-accumulated; d_t row 0, -lap row 32; tail adds c2*acc32)

Engine split: ACT = tanh + jets squares (PSUM drains); DVE = stt gating
(vt/hj), mall products, tail; GpSimd = h^2 and r = h*vtn.  Jets run as two
independent 2-direction pipelines with double-buffered [W,2,C] PSUM tiles.
Chunks sequential (fwd(c+1) ACT/PE work naturally overlaps jets(c) DVE work);
first chunks small to fill the pipeline early.  Input-layer matmuls use f32r
(1 cycle/row).  Heartbeat matmuls keep the PE activity monitor warm, writing
unused partitions 64:96 of the chunk's acc bank.
Sharding: data parallel, 16384 points -> 8 cores x 2048.
"""

import sys

sys.path.insert(0, "/opt/trn_rl_repo")

from contextlib import ExitStack

import numpy as np

import concourse.bacc as bacc
import concourse.bass as bass
import concourse.tile as tile
from concourse import mybir
from concourse.bass_utils import run_bass_kernel_spmd

N_CORES = 8
NPTS = 2048  # points per core
CHUNKS = [512, 512, 512, 512]  # sum = NPTS; 4 chunks beat 5/6 (per-chunk overhead)
MALL_ON_POOL = set()  # POOL mall measured ~3x DVE cost; keep malls on DVE
MAXC = 512
W = 128  # WavePINN width
NHID = 5
NLAY = 6
PW = 32  # psi width

F32 = mybir.dt.float32
F32R = mybir.dt.float32r
BF16 = mybir.dt.bfloat16
AF = mybir.ActivationFunctionType
ALU = mybir.AluOpType


def build_nc():
    nc = bacc.Bacc()

    pts = nc.declare_dram_parameter("pts", [4, NPTS], F32R, isOutput=False)
    # bundled weights: fewer DMA dispatches at startup
    wpf = nc.declare_dram_parameter("wpf", [W, 11], F32, isOutput=False)
    wpb = nc.declare_dram_parameter("wpb", [W, 2 * NHID * W + 4 + 4 * W], BF16, isOutput=False)
    ppack = nc.declare_dram_parameter("ppack", [4, W + PW], F32R, isOutput=False)
    pwpack = nc.declare_dram_parameter("pwpack", [PW, PW + 1], BF16, isOutput=False)
    pbias3 = nc.declare_dram_parameter("pbias3", [PW, 3], F32, isOutput=False)
    resid = nc.declare_dram_parameter("resid", [1, NPTS], F32, isOutput=True)

    offs = []
    o = 0
    for cl in CHUNKS:
        offs.append(o)
        o += cl

    with tile.TileContext(nc) as tc, ExitStack() as ctx:
        const = ctx.enter_context(tc.tile_pool(name="const", bufs=1))
        acts = ctx.enter_context(tc.tile_pool(name="acts", bufs=5))
        work = ctx.enter_context(tc.tile_pool(name="work", bufs=2))
        jwork = ctx.enter_context(tc.tile_pool(name="jwork", bufs=4))
        mpool = ctx.enter_context(tc.tile_pool(name="mpool", bufs=5))
        ps_f = ctx.enter_context(tc.tile_pool(name="ps_f", bufs=2, space="PSUM"))
        ps_j = ctx.enter_context(tc.tile_pool(name="ps_j", bufs=2, space="PSUM"))
        ps_d = ctx.enter_context(tc.tile_pool(name="ps_d", bufs=2, space="PSUM"))

        # ---- const loads spread over engine queues (each queue serializes
        # its own DMAs); pts split per-chunk so chunk 0 starts early ----
        ppack_sb = const.tile([4, W + PW], F32R, tag="ppack")
        nc.gpsimd.dma_start(out=ppack_sb[:], in_=ppack[:])
        wpb_sb = const.tile([W, 2 * NHID * W + 4 + 4 * W], BF16, tag="wpb")
        nc.gpsimd.dma_start(out=wpb_sb[:], in_=wpb[:])
        wpf_sb = const.tile([W, 11], F32, tag="wpf")
        nc.scalar.dma_start(out=wpf_sb[:], in_=wpf[:])
        pwpack_sb = const.tile([PW, PW + 1], BF16, tag="pwpack")
        nc.scalar.dma_start(out=pwpack_sb[:], in_=pwpack[:])
        pts_sb = const.tile([4, NPTS], F32R, tag="pts")
        for c, cl in enumerate(CHUNKS):
            nc.sync.dma_start(
                out=pts_sb[:, offs[c] : offs[c] + cl],
                in_=pts[:, offs[c] : offs[c] + cl],
            )
        pbias3_sb = const.tile([PW, 3], F32, tag="pbias3")
        nc.sync.dma_start(out=pbias3_sb[:], in_=pbias3[:])

        bias_sb = wpf_sb[:, 0:NLAY]
        wout_sb = wpf_sb[:, NLAY : NLAY + 1]
        w1cols_sb = wpf_sb[:, NLAY + 1 : NLAY + 5]
        wfwd_sb = wpb_sb[:, 0 : NHID * W]
        wbwd_sb = wpb_sb[:, NHID * W : 2 * NHID * W]
        jl2_sb = wpb_sb[:, 2 * NHID * W : 2 * NHID * W + 2]
        ones2_sb = wpb_sb[:, 2 * NHID * W + 2 : 2 * NHID * W + 4]
        wjf_sb = wpb_sb[:, 2 * NHID * W + 4 : 2 * NHID * W + 4 + 4 * W]
        w1t_sb = ppack_sb[:, 0:W]
        pw1t_sb = ppack_sb[:, W : W + PW]
        pw2t_sb = pwpack_sb[:, 0:PW]
        pwot_sb = pwpack_sb[:, PW : PW + 1]
        pb1_sb = pbias3_sb[0:1, 2:3]

        def wf(k):  # fwd lhsT for 0-idx layer k (1..5)
            return wfwd_sb[:, (k - 1) * W : k * W]

        def wb(k):  # bwd lhsT
            return wbwd_sb[:, (k - 1) * W : k * W]

        # contraction columns: ones2 = [+2 | -2]
        def wcol_d0(k):  # d_t row weight for layer k (1..5)
            return ones2_sb[:, 0:1] if k % 2 == 1 else ones2_sb[:, 1:2]

        def wcol_lap(k):  # -lap row weight
            return ones2_sb[:, 1:2] if k % 2 == 1 else ones2_sb[:, 0:1]

        # ---- warmup: pull the ACT table load + PE clock ramp off the
        # critical path while the input DMAs are in flight ----
        wz = work.tile([W, MAXC], BF16, tag="wz")
        nc.gpsimd.memset(wz, 0)
        wrm = work.tile([1, 1], F32, tag="wrm")
        nc.gpsimd.memset(wrm, 0)
        nc.scalar.activation(wrm, wrm, AF.Tanh)
        for _ in range(2):
            wu_ps = ps_f.tile([W, MAXC], F32, tag="a", name="wu_ps")
            nc.tensor.matmul(wu_ps, wz[:, 0:W], wz, start=True, stop=True)

        def heartbeat(lhsT, rhs, acc_ps):
            # keep the PE activity monitor warm: dummy matmul tied to a
            # freshly produced tensor so the scheduler spreads them in time;
            # writes unused partitions 64:96 of the chunk's acc bank
            nc.tensor.matmul(
                acc_ps[64:96, 0:256], lhsT[:, 0:32], rhs[:, 0:256],
                start=True, stop=True, skip_group_check=True,
            )

        for c, cl in enumerate(CHUNKS):
            sl = slice(offs[c], offs[c] + cl)

            y_sb = acts.tile([W, NLAY, MAXC], BF16, tag="y", name="y")
            sq_sb = acts.tile([W, NLAY, MAXC], BF16, tag="sq", name="sq")
            r_sb = acts.tile([W, NLAY, MAXC], BF16, tag="r", name="r")
            acc_ps = ps_d.tile([W, MAXC], F32, tag="acc", name="acc")

            # ---- forward ----
            for k in range(NLAY):
                a_ps = ps_f.tile([W, MAXC], F32, tag="a", name="a_ps")
                if k == 0:
                    # f32r: ~fp32 input precision at 1 cycle/row (vs 4 for f32)
                    nc.tensor.matmul(
                        a_ps[:, :cl], w1t_sb, pts_sb[:, sl], start=True, stop=True
                    )
                else:
                    nc.tensor.matmul(
                        a_ps[:, :cl], wf(k), y_sb[:, k - 1, :cl],
                        start=True, stop=True,
                    )
                nc.scalar.activation(
                    y_sb[:, k, :cl], a_ps[:, :cl], AF.Tanh,
                    bias=bias_sb[:, k : k + 1],
                )
                nc.gpsimd.tensor_tensor(
                    sq_sb[:, k, :cl], y_sb[:, k, :cl], y_sb[:, k, :cl], ALU.mult
                )
                if k == 0:
                    # d1n = sq_1 - 1 (folded layer-2 jet rhs); hoisted here so
                    # the jets matmuls are not gated behind the bwd vt chain
                    # in the DVE queue
                    d1n = work.tile([W, MAXC], BF16, tag="d1n", name="d1n")
                    nc.vector.tensor_scalar(
                        d1n[:, :cl], sq_sb[:, 0, :cl], 1.0, -1.0,
                        ALU.mult, ALU.add,
                    )

            # ---- psi network (emitted early so its ACT ops land in the
            # forward phase, keeping ACT free during jets) ----
            pp_ps = ps_f.tile([PW, MAXC], F32, tag="a", name="pp_ps")
            nc.tensor.matmul(
                pp_ps[:, :cl], pw1t_sb, pts_sb[:, sl], start=True, stop=True
            )
            hp1 = work.tile([PW, MAXC], BF16, tag="hp", name="hp1")
            nc.scalar.activation(
                hp1[:, :cl], pp_ps[:, :cl], AF.Tanh, bias=pbias3_sb[:, 0:1]
            )
            pp2_ps = ps_f.tile([PW, MAXC], F32, tag="a", name="pp2_ps")
            nc.tensor.matmul(
                pp2_ps[:, :cl], pw2t_sb, hp1[:, :cl], start=True, stop=True
            )
            hp2 = work.tile([PW, MAXC], BF16, tag="hp", name="hp2")
            nc.scalar.activation(
                hp2[:, :cl], pp2_ps[:, :cl], AF.Tanh, bias=pbias3_sb[:, 1:2]
            )
            psi_ps = ps_f.tile([1, MAXC], F32, tag="a", name="psi_ps")
            nc.tensor.matmul(
                psi_ps[:, :cl], pwot_sb, hp2[:, :cl], start=True, stop=True
            )
            c2 = work.tile([1, MAXC], F32, tag="c2", name="c2")
            nc.scalar.activation(c2[:, :cl], psi_ps[:, :cl], AF.Square, bias=pb1_sb)

            # ---- backward (negated-adjoint recurrence) ----
            vt = work.tile([W, MAXC], BF16, tag="vt", name="vt")
            nc.vector.scalar_tensor_tensor(
                vt[:, :cl], sq_sb[:, NLAY - 1, :cl], 1.0,
                wout_sb.to_broadcast((W, cl)),
                ALU.subtract, ALU.mult,
            )
            nc.gpsimd.tensor_tensor(
                r_sb[:, NLAY - 1, :cl], y_sb[:, NLAY - 1, :cl], vt[:, :cl], ALU.mult
            )
            hb2 = None  # bwd heartbeat deferred one step: emitted after the
            # next bwd matmul so it fills that matmul's vt-wait gap instead of
            # adding serial PE time to the DVE-paced bwd chain
            for k in range(NLAY - 1, 0, -1):
                v_ps = ps_f.tile([W, MAXC], F32, tag="a", name="v_ps")
                nc.tensor.matmul(
                    v_ps[:, :cl], wb(k), vt[:, :cl], start=True, stop=True
                )
                if hb2 is not None:
                    heartbeat(*hb2)
                vt = work.tile([W, MAXC], BF16, tag="vt", name="vt")
                nc.vector.scalar_tensor_tensor(
                    vt[:, :cl], sq_sb[:, k - 1, :cl], 1.0, v_ps[:, :cl],
                    ALU.subtract, ALU.mult,
                )
                hb2 = (wb(k), vt[:, :cl], acc_ps)
                nc.gpsimd.tensor_tensor(
                    r_sb[:, k - 1, :cl], y_sb[:, k - 1, :cl], vt[:, :cl], ALU.mult
                )

            # ---- jets + curvature contraction ----
            # acc row 0 = d_t, row 32 = -lap (matmul outs need base part 0/32)
            # two independent direction-pair pipelines: half 0 = (t,x), 1 = (y,z)
            hjh = [None, None]
            pend = []  # deferred contraction matmuls (PE head-of-line relief)
            hb_pend = []  # deferred jets heartbeats (same reason)
            mall_pend = []  # malls wait on ACT's sqj; deferred one half-step
            # so the next half's ready hj gating is never queued behind them

            def emit_mall(pk, ph, psqj):
                mall = mpool.tile([W, 2, MAXC], BF16, tag="mall", name="mall")
                rbc = r_sb[:, pk, :cl].unsqueeze(1).to_broadcast((W, 2, cl))
                nc.vector.tensor_tensor(
                    mall[:, :, :cl], psqj[:, :, :cl], rbc, ALU.mult
                )
                pend.append((pk, ph, mall))
            jl2_pend = True  # jl2 waits on r[0] (last bwd product); deferred
            # past the ready aj(1) matmuls so it doesn't block them

            def contract(k, h, mall, do_stop):
                if h == 0:
                    nc.tensor.matmul(
                        acc_ps[0:1, :cl], wcol_d0(k), mall[:, 0, :cl],
                        start=False, stop=do_stop, skip_group_check=True,
                    )
                    nc.tensor.matmul(
                        acc_ps[32:33, :cl], wcol_lap(k), mall[:, 1, :cl],
                        start=False, stop=False, skip_group_check=True,
                    )
                else:
                    for i in range(2):
                        nc.tensor.matmul(
                            acc_ps[32:33, :cl], wcol_lap(k), mall[:, i, :cl],
                            start=False, stop=(do_stop and i == 1),
                            skip_group_check=True,
                        )

            for k in range(1, NLAY):
                last = k == NLAY - 1
                for h in range(2):
                    aj_ps = ps_j.tile([W, 2, MAXC], F32, tag="aj", name="aj_ps")
                    for i in range(2):
                        if k == 1:
                            nc.tensor.matmul(
                                aj_ps[:, i, :cl],
                                wjf_sb[:, (2 * h + i) * W : (2 * h + i + 1) * W],
                                d1n[:, :cl], start=True, stop=True,
                            )
                        else:
                            nc.tensor.matmul(
                                aj_ps[:, i, :cl], wf(k), hjh[h][:, i, :cl],
                                start=True, stop=True,
                            )
                    if jl2_pend:
                        jl2_pend = False
                        nc.tensor.matmul(
                            acc_ps[0:1, :cl], jl2_sb[:, 0:1], r_sb[:, 0, :cl],
                            start=True, stop=False, skip_group_check=True,
                        )
                        nc.tensor.matmul(
                            acc_ps[32:33, :cl], jl2_sb[:, 1:2], r_sb[:, 0, :cl],
                            start=True, stop=False, skip_group_check=True,
                        )
                    if pend:
                        contract(*pend.pop(), do_stop=False)
                    if hb_pend:
                        heartbeat(*hb_pend.pop())
                    sqj = jwork.tile([W, 2, MAXC], BF16, tag="sqj", name="sqj")
                    nc.scalar.activation(
                        sqj[:, :, :cl], aj_ps[:, :, :cl], AF.Square
                    )
                    # deferred one half-step: a heartbeat waits on its rhs
                    # (ACT's sqj), so emitting it immediately would block the
                    # other half's ready aj matmuls behind an ACT dependency
                    hb_pend.append((wf(k), sqj[:, 0, :cl], acc_ps))
                    if not last:
                        hj = jwork.tile([W, 2, MAXC], BF16, tag="hj", name="hj")
                        sqbc = sq_sb[:, k, :cl].unsqueeze(1).to_broadcast((W, 2, cl))
                        nc.vector.scalar_tensor_tensor(
                            hj[:, :, :cl], sqbc, 1.0, aj_ps[:, :, :cl],
                            ALU.subtract, ALU.mult,
                        )
                        hjh[h] = hj
                    if mall_pend:
                        emit_mall(*mall_pend.pop(0))
                    mall_pend.append((k, h, sqj))
            while mall_pend:
                emit_mall(*mall_pend.pop(0))
            while pend:
                contract(*pend.pop(0), do_stop=(not pend))

            # ---- tail: resid = d_t + c2*(-lap) ----
            m1 = work.tile([1, MAXC], F32, tag="m1", name="m1")
            nc.vector.tensor_tensor(
                m1[:, :cl], c2[:, :cl], acc_ps[32:33, :cl], ALU.mult
            )
            res_sb = work.tile([1, MAXC], F32, tag="res", name="res_sb")
            nc.vector.tensor_tensor(
                res_sb[:, :cl], m1[:, :cl], acc_ps[0:1, :cl], ALU.add
            )
            nc.sync.dma_start(out=resid[0:1, sl], in_=res_sb[:, :cl])

    return nc


_NC_CACHE = {}


def _get_nc():
    if "nc" not in _NC_CACHE:
        nc = build_nc()
        nc.finalize()
        _NC_CACHE["nc"] = nc
    return _NC_CACHE["nc"]


def _bf(a):
    import ml_dtypes

    return np.asarray(a, np.float32).astype(ml_dtypes.bfloat16)


def _r32r(a):
    # pre-round to fp32r (keep 13 explicit mantissa bits) as walrus requires
    x = np.ascontiguousarray(np.asarray(a, np.float32))
    u = x.view(np.uint32).copy()
    u &= np.uint32(0xFFFFFFFF) << np.uint32(10)
    return u.view(np.float32)


def make_in_maps(t, x, y, z, uW_in, ub_in, uW_hid, ub_hid, uW_out, ub_out,
                 pW_in, pb_in, pW_hid, pb_hid, pW_out, pb_out):
    f = lambda a: np.ascontiguousarray(np.asarray(a, np.float32))
    uW_in, ub_in, uW_hid, ub_hid = f(uW_in), f(ub_in), f(uW_hid), f(ub_hid)
    uW_out, pW_in, pb_in = f(uW_out), f(pW_in), f(pb_in)
    pW_hid, pb_hid, pW_out, pb_out = f(pW_hid), f(pb_hid), f(pW_out), f(pb_out)

    pts_full = np.stack([f(t), f(x), f(y), f(z)], axis=0)  # [4, 16384]

    # wpf: biases [W,6] | wout [W,1] | w1cols [W,4]
    biases = np.concatenate([ub_in[:, None], ub_hid.T], axis=1)
    wpf = np.concatenate([biases, uW_out[0][:, None], uW_in], axis=1)
    # wpb: wfwd [W,640] | wbwd [W,640] | jl2 [W,2] | ones2 [W,2]
    wfwd = np.concatenate([uW_hid[i].T for i in range(NHID)], axis=1)
    wbwd = np.concatenate([uW_hid[i] for i in range(NHID)], axis=1)
    # d_t row gets -2*W1t^2; -lap row gets +2*sum(W1xyz^2)
    jl2 = np.stack(
        [-2.0 * uW_in[:, 0] ** 2, 2.0 * (uW_in[:, 1:4] ** 2).sum(1)], axis=1
    )
    ones2 = np.concatenate(
        [2.0 * np.ones([W, 1], np.float32), -2.0 * np.ones([W, 1], np.float32)],
        axis=1,
    )
    # wjf: 4 folded layer-2 jet matrices, lhsT[j,a] = W2[a,j]*W1[j,i]
    wjf = np.concatenate(
        [uW_hid[0].T * uW_in[:, i : i + 1] for i in range(4)], axis=1
    )
    wpb = np.concatenate([wfwd, wbwd, jl2, ones2, wjf], axis=1)
    # ppack: w1t [4,128] | pw1t [4,32]
    ppk = np.concatenate([uW_in.T, pW_in.T], axis=1)
    # pwpack: pw2t [32,32] | pwot [32,1]
    pwp = np.concatenate([pW_hid[0].T, pW_out[0][:, None]], axis=1)
    # pbias3: pb_in | pb_hid[0] | (pb_out+1 at row 0)
    pb3 = np.zeros([PW, 3], np.float32)
    pb3[:, 0] = pb_in
    pb3[:, 1] = pb_hid[0]
    pb3[0, 2] = pb_out[0] + 1.0

    shared = dict(
        wpf=f(wpf),
        wpb=_bf(wpb),
        ppack=_r32r(ppk),
        pwpack=_bf(pwp),
        pbias3=f(pb3),
    )
    in_maps = []
    for cid in range(N_CORES):
        m = dict(shared)
        m["pts"] = _r32r(pts_full[:, cid * NPTS : (cid + 1) * NPTS])
        in_maps.append(m)
    return in_maps


def kernel(**inputs):
    in_maps = make_in_maps(**inputs)
    nc = _get_nc()
    res = run_bass_kernel_spmd(nc, in_maps, list(range(N_CORES))).results
    out = np.concatenate(
        [np.asarray(res[cid]["resid"]).reshape(-1) for cid in range(N_CORES)]
    )
    return out.astype(np.float32)


if __name__ == "__main__":
    nc = build_nc()
    nc.finalize()
    print("built ok:", nc)
